# revision 4
# baseline (speedup 1.0000x reference)
"""Trainium2 Bass kernel for a dense transformer block (pre-LN, causal MHA + FFN).

Reference computation (per batch element b, T=64 tokens, D=384 features):
    h   = LN(x)*g1 + be1
    q,k,v per-head linears; scores = q k^T / sqrt(48); causal softmax
    attn = probs @ v, concat heads, @ wo + bo
    h    = h + attn              (residual from the *normed* x)
    h2   = LN(h)*g2 + be2
    out  = h2 + relu(h2@w1+b1)@w2 + b2

Sharding: pure data parallel over batch (2048 -> 256 per core, 8 cores),
params replicated; the same single-core program runs SPMD on all 8 cores.

v2 design (vs the f32r baseline):
  - all matmul operands bf16: 1 cyc/row on PE regardless of free size
    (f32r needs free>=256; fp32 is 4 cyc/row), and transposes at 1 cyc/row.
  - attention computes scores TRANSPOSED: scT[s,q] = K Q^T per (batch-pair,
    head) with lhsT=K^T / rhs=Q^T straight from the feature-major QK
    projection - no probs transpose at all.  exp+causal/block mask in
    [s_part, q_free] layout; row sums arrive via a ones-column smuggled into
    V's 64-padding, so attnV emits [q, 48 attn | sum] per head; one
    stride-0-broadcast tensor_tensor both normalizes and packs heads to
    [q, 384].  Per (pair,head): 1 scores mm + 1 attnV mm (vs 4 small mms +
    transpose in the baseline).
  - LN rstd = exp(-0.5*ln(var+eps)): ln+exp+relu+copy live in ONE activation
    table (sqrt does not), killing 2 ACT_TABLE_LOADs (1.3us each) per tile.
  - PSUM->SBUF copies batched per bank; pool/gpsimd engine takes the
    transpose copies; software pipeline runs FFN(i-1) matmuls inside
    attention(i)'s cross-engine stalls to keep PE p-state high.
"""

import os
import sys

sys.path.insert(0, "/opt/trn_rl_repo")

import numpy as np

import concourse.bass as bass
import concourse.tile as tile
from concourse import mybir

# ---- problem constants (hardcoded per contract) ----
B_TOTAL = 2048
T = 64
D = 384
H = 8
E = 48  # head size
EP = 64  # padded head size
F = 4 * D  # ffn hidden 1536
N_CORES = 8
B_CORE = B_TOTAL // N_CORES  # 256
LN_EPS = 1e-5
INV_SQRT_E = float(E) ** -0.5

NB = 4  # batch elems per tile
NT = NB * T  # tokens per tile = 256
KC = D // 128  # 3 contraction chunks for D
FC = F // 128  # 12 chunks for ffn hidden
TC = NT // 128  # 2 token chunks per tile
NP = NB // 2  # batch pairs per tile = 2
DP = H * EP  # padded v width 512

F32 = mybir.dt.float32
BF16 = mybir.dt.bfloat16

PIPELINE = os.environ.get("KPIPE", "1") == "1"
ACT_TRICK = os.environ.get("KACTTRICK", "1") == "1"
LN_ON_ACT = os.environ.get("KLNACT", "1") == "1"
STRIDE0 = os.environ.get("KSTRIDE0", "1") == "1"
BANK2 = os.environ.get("KBANK2", "1") == "1"
MASK_INPLACE = os.environ.get("KMASKIP", "1") == "1"
CUT = float(os.environ.get("KCUT", "7"))  # sequential-mode stage bisect


def build_body(tc, aps, b_core):
    from contextlib import ExitStack

    ctx = ExitStack()
    nc = tc.nc
    n_tiles = b_core * T // NT

    x_dr = aps["x"].rearrange("b t d -> (b t) d")
    out_dr = aps["out"].rearrange("b t d -> (b t) d")

    AF = mybir.ActivationFunctionType
    OP = mybir.AluOpType
    flags = aps["flags"]

    singles = ctx.enter_context(tc.tile_pool(name="singles", bufs=1))

    def load_const(name, shape, src_ap, dt=BF16):
        t_ = singles.tile(list(shape), dt, name=f"sb_{name}")
        nc.sync.dma_start(out=t_, in_=src_ap)
        return t_

    ident = load_const("ident", [128, 128], aps["ident"])
    maskT = load_const("maskT", [128, H * 128], aps["maskT"])
    wqk = {
        (qi, k, ch): load_const(f"wqk{qi}{k}{ch}", [128, 128], aps["wqk"][qi, k, ch])
        for qi in range(2)
        for k in range(KC)
        for ch in range(4)
    }
    wv = {k: load_const(f"wv{k}", [128, DP], aps["wv"][k]) for k in range(KC)}
    wo = {k: load_const(f"wo{k}", [128, D], aps["wo"][k]) for k in range(KC)}
    w1 = {
        (k, f): load_const(f"w1{k}_{f}", [128, 128], aps["w1"][k, f])
        for k in range(KC)
        for f in range(FC)
    }
    w2 = {f: load_const(f"w2{f}", [128, D], aps["w2"][f]) for f in range(FC)}
    bqk = load_const("bqk", [128, 8], aps["bqk"], F32)
    bv_b = load_const("bv_b", [128, DP], aps["bv_b"], F32)
    b1c = load_const("b1c", [128, FC], aps["b1c"], F32)
    g1_b = load_const("g1_b", [128, D], aps["g1_b"], F32)
    be1_b = load_const("be1_b", [128, D], aps["be1_b"], F32)
    g2_b = load_const("g2_b", [128, D], aps["g2_b"], F32)
    be2_b = load_const("be2_b", [128, D], aps["be2_b"], F32)
    bo_b = load_const("bo_b", [128, D], aps["bo_b"], F32)
    b2_b = load_const("b2_b", [128, D], aps["b2_b"], F32)

    eps_t = singles.tile([128, 1], F32, name="eps")
    nc.vector.memset(eps_t, LN_EPS)

    pool = lambda nm, n, **kw: ctx.enter_context(tc.tile_pool(name=nm, bufs=n, **kw))
    ps = pool("ps", 4, space="PSUM")  # 1-bank tiles ring
    ps2 = pool("ps2", 2, space="PSUM")  # 2-bank [128,1024]f32 tiles ring
    # NOTE: pools holding TC=2 chunk tiles per tile-iteration need bufs in
    # CHUNK units: 2 live tiles across the pipeline = 4 chunks + slack.
    p_x = pool("p_x", 4)
    p_h = pool("p_h", 6)
    p_hT = pool("p_hT", 2)
    p_qk = pool("p_qk", 2)  # 2 tags
    p_v = pool("p_v", 2)  # 2 tags
    p_ex = pool("p_ex", 2)  # 2 tags
    p_at = pool("p_at", 2)  # 2 tags
    p_aT = pool("p_aT", 2)
    p_hr = pool("p_hr", 4)
    p_h2 = pool("p_h2", 4)
    p_h2T = pool("p_h2T", 2)
    p_rel = pool("p_rel", 2)
    p_out = pool("p_out", 4)
    p_st = pool("p_st", 6)

    def layernorm(x_t, g_b, be_b, gb_nontriv, pool_, tag):
        """Token-major LN -> bf16. rstd = exp(-0.5*ln(var+eps)) keeps the
        activation table fixed (ln/exp/relu/copy share one table; sqrt does
        not)."""
        st = p_st.tile([128, 6], F32, tag="st", name=f"st_{tag}")
        nc.vector.bn_stats(out=st, in_=x_t)
        mv = p_st.tile([128, 2], F32, tag="mv", name=f"mv_{tag}")
        nc.vector.bn_aggr(out=mv, in_=st)
        mean, var = mv[:, 0:1], mv[:, 1:2]
        lnv = p_st.tile([128, 1], F32, tag="ln", name=f"ln_{tag}")
        nc.scalar.activation(out=lnv, in_=var, func=AF.Ln, bias=eps_t, scale=1.0)
        rstd = p_st.tile([128, 1], F32, tag="rs", name=f"rs_{tag}")
        nc.scalar.activation(out=rstd, in_=lnv, func=AF.Exp, bias=0.0, scale=-0.5)
        nmr = p_st.tile([128, 1], F32, tag="nm", name=f"nm_{tag}")
        nc.vector.scalar_tensor_tensor(
            out=nmr, in0=mean, scalar=-1.0, in1=rstd, op0=OP.mult, op1=OP.mult
        )
        h_t = pool_.tile([128, D], BF16, tag=tag, name=f"h_{tag}")
        if LN_ON_ACT:
            # normalize fused on Act: Identity(rstd*x + nmr), per-partition APs
            nc.scalar.activation(
                out=h_t, in_=x_t, func=AF.Identity, bias=nmr, scale=rstd
            )
        else:
            nc.vector.tensor_scalar(
                out=h_t, in0=x_t, scalar1=rstd, scalar2=nmr, op0=OP.mult, op1=OP.add
            )
        if gb_nontriv:
            nc.vector.tensor_tensor(out=h_t, in0=h_t, in1=g_b, op=OP.mult)
            nc.vector.tensor_tensor(out=h_t, in0=h_t, in1=be_b, op=OP.add)
        return h_t

    def transpose_feat(h_ts, dst_pool, tag):
        """token-major [128, D] x TC (bf16) -> feature-major [128, KC*NT]
        (col k*NT + tok). All TC*KC transposes land in ONE 1-bank bf16 psum
        tile; a single strided DVE copy (bf16 in+out -> 2x mode) moves it."""
        hT = dst_pool.tile([128, KC * NT], BF16, tag=tag, name=f"hT_{tag}")
        hT3 = hT.rearrange("p (k t) -> p k t", k=KC)
        pt_ = ps.tile([128, TC * D], BF16, tag="ps", name=f"tp_{tag}")
        for c in range(TC):
            for k in range(KC):
                nc.tensor.transpose(
                    out=pt_[:, (c * KC + k) * 128 : (c * KC + k) * 128 + 128],
                    in_=h_ts[c][:, k * 128 : (k + 1) * 128],
                    identity=ident,
                )
        src4 = pt_.rearrange("p (c k t) -> p k c t", c=TC, k=KC)
        nc.vector.tensor_copy(out=hT3, in_=src4)
        return hT

    # ---------- stage: load x + LN1 (no PE work; runs a tile ahead) ----------
    def stage_ln1(it):
        row0 = it * NT
        st = {}
        x_ts = []
        for c in range(TC):
            x_t = p_x.tile([128, D], F32, tag="x", name="x")
            nc.sync.dma_start(
                out=x_t, in_=x_dr[row0 + c * 128 : row0 + (c + 1) * 128, :]
            )
            x_ts.append(x_t)
        st["h"] = [
            layernorm(x_ts[c], g1_b, be1_b, flags["g1be1"], p_h, "h") for c in range(TC)
        ]
        return st

    # ---------- stage: hT transposes + QKV projections ----------
    def stage_qkv(st):
        hT = transpose_feat(st["h"], p_hT, "hT")

        # Q,K feature-major [128 = 2 heads 64-padded, NT], ch-major cols.
        qk_sb = []
        for qi in range(2):
            sb = p_qk.tile([128, 4 * NT], BF16, tag=f"qk{qi}", name=f"qk{qi}")
            if BANK2:
                pms = [ps2.tile([128, 4 * NT], F32, tag="ps2", name="qk_ps")]
                pmof = [(pms[0], 0)] * 4
            else:
                pms = [ps.tile([128, 2 * NT], F32, tag="ps", name="qk_ps") for _ in range(2)]
                pmof = [(pms[0], 0), (pms[0], 0), (pms[1], 2 * NT), (pms[1], 2 * NT)]
            pm = pms[0]
            for ch in range(4):
                tgt, off = pmof[ch]
                for k in range(KC):
                    nc.tensor.matmul(
                        out=tgt[:, ch * NT - off : (ch + 1) * NT - off],
                        lhsT=wqk[(qi, k, ch)],
                        rhs=hT[:, k * NT : (k + 1) * NT],
                        start=(k == 0),
                        stop=(k == KC - 1),
                    )
            if not BANK2 and not flags["bqk"]:
                for i2, pmx in enumerate(pms):
                    if qi == 0:
                        nc.scalar.copy(out=sb[:, i2 * 2 * NT : (i2 + 1) * 2 * NT], in_=pmx)
                    else:
                        nc.vector.tensor_copy(out=sb[:, i2 * 2 * NT : (i2 + 1) * 2 * NT], in_=pmx)
                qk_sb.append(sb)
                continue
            if flags["bqk"]:
                for ch in range(4):
                    tgt, off = pmof[ch]
                    nc.scalar.activation(
                        out=sb[:, ch * NT : (ch + 1) * NT],
                        in_=tgt[:, ch * NT - off : (ch + 1) * NT - off],
                        func=AF.Identity,
                        bias=bqk[:, qi * 4 + ch : qi * 4 + ch + 1],
                        scale=1.0,
                    )
            elif qi == 0:
                nc.scalar.copy(out=sb, in_=pm)
            else:
                nc.vector.tensor_copy(out=sb, in_=pm)
            lo = p_qk.tile([64, 4 * NT], BF16, tag=f"qlo{qi}", name=f"qlo{qi}")
            nc.sync.dma_start(out=lo[0:48, :], in_=sb[64:112, :])
            qk_sb.append((sb, lo))
        st["qk"] = qk_sb

        # V token-major [128 tok of pair p, DP] with a ones column smuggled
        # at h*64+48 (inside the zero padding) for softmax row sums.
        v_sb = []
        for p in range(NP):
            pm = ps.tile([128, DP], F32, tag="ps", name="v_ps")
            for k in range(KC):
                nc.tensor.matmul(
                    out=pm,
                    lhsT=hT[:, k * NT + p * 128 : k * NT + (p + 1) * 128],
                    rhs=wv[k],
                    start=(k == 0),
                    stop=(k == KC - 1),
                )
            sb = p_v.tile([128, DP], BF16, tag=f"v{p}", name=f"v{p}")
            sb3 = sb.rearrange("p (h e) -> p h e", h=H)
            pm3 = pm.rearrange("p (h e) -> p h e", h=H)
            if flags["bv"]:
                nc.vector.tensor_tensor(out=sb, in0=pm, in1=bv_b, op=OP.add)
            else:
                nc.scalar.copy(out=sb3[:, :, 0:E], in_=pm3[:, :, 0:E])
            nc.gpsimd.memset(sb3[:, :, E], 1.0)
            v_sb.append(sb)
        st["v"] = v_sb
        return st

    # ---------- scores + softmax numerator for one batch pair ----------
    def stage_scores_pair(st, p):
        k_sb, q_sb = st["qk"][1], st["qk"][0]
        if BANK2:
            scs_t = ps2.tile([128, H * 128], F32, tag="ps2", name="sc")
            sco = [(scs_t, 0)] * 8
        else:
            sa = ps.tile([128, 4 * 128], F32, tag="ps", name="sca")
            sb2 = ps.tile([128, 4 * 128], F32, tag="ps", name="scb")
            sco = [(sa, 0)] * 4 + [(sb2, 512)] * 4
        for h in range(H):
            # even heads live at partitions 0:48 of the projection tile; odd
            # heads were DMA-repacked to partitions 0:48 of the `lo` tile so
            # every matmul runs at tile_position (0,0) - alternating PE
            # partition bases between matmuls wedges the device.
            ksrc = k_sb[h % 2]
            qsrc = q_sb[h % 2]
            ch = h // 2
            sl = slice(ch * NT + p * 128, ch * NT + (p + 1) * 128)
            tgt, o0 = sco[h]
            nc.tensor.matmul(
                out=tgt[:, h * 128 - o0 : (h + 1) * 128 - o0],
                lhsT=ksrc[0:E, sl],
                rhs=qsrc[0:E, sl],
                start=True,
                stop=True,
            )
        ex = p_ex.tile([128, H * 128], BF16, tag=f"ex{p}", name=f"ex{p}")
        if CUT < 2.5:
            # drain psum with a plain copy; no exp/mask
            nc.vector.tensor_copy(out=ex, in_=scs_t if BANK2 else sa)
            st.setdefault("ex", []).append(ex)
            return
        if BANK2:
            nc.scalar.activation(
                out=ex, in_=scs_t, func=AF.Exp, bias=0.0, scale=INV_SQRT_E
            )
        else:
            nc.scalar.activation(
                out=ex[:, 0:512], in_=sa, func=AF.Exp, bias=0.0, scale=INV_SQRT_E
            )
            nc.scalar.activation(
                out=ex[:, 512:1024], in_=sb2, func=AF.Exp, bias=0.0, scale=INV_SQRT_E
            )
        if CUT < 2.9:
            st.setdefault("ex", []).append(ex)
            return
        if MASK_INPLACE:
            nc.vector.tensor_tensor(out=ex, in0=ex, in1=maskT, op=OP.mult)
            st.setdefault("ex", []).append(ex)
        else:
            ex2 = p_ex.tile([128, H * 128], BF16, tag=f"exm{p}", name=f"exm{p}")
            nc.vector.tensor_tensor(out=ex2, in0=ex, in1=maskT, op=OP.mult)
            st.setdefault("ex", []).append(ex2)

    # ---------- attnV + normalize for one batch pair ----------
    def stage_atv_pair(st, p):
        ex, v = st["ex"][p], st["v"][p]
        atp = ps.tile([128, DP], F32, tag="ps", name="at_ps")
        for h in range(H):
            nc.tensor.matmul(
                out=atp[:, h * EP : h * EP + E + 1],
                lhsT=ex[:, h * 128 : (h + 1) * 128],
                rhs=v[:, h * EP : h * EP + E + 1],
                start=True,
                stop=True,
            )
        atp3 = atp.rearrange("p (h e) -> p h e", h=H)
        rr = p_st.tile([128, H], F32, tag="rr", name="rr")
        nc.vector.reciprocal(out=rr, in_=atp3[:, :, E])
        sb = p_at.tile([128, H * E], BF16, tag=f"at{p}", name=f"at{p}")
        if STRIDE0:
            nc.vector.tensor_tensor(
                out=sb.rearrange("p (h e) -> p h e", h=H),
                in0=atp3[:, :, 0:E],
                in1=rr.broadcast_to([128, H, E]),
                op=OP.mult,
            )
        else:
            for h in range(H):
                nc.vector.tensor_scalar(
                    out=sb[:, h * E : (h + 1) * E],
                    in0=atp[:, h * EP : h * EP + E],
                    scalar1=rr[:, h : h + 1],
                    scalar2=None,
                    op0=OP.mult,
                )
        st.setdefault("at", []).append(sb)

    # ---------- attnT transposes (end of the tile's emission) ----------
    def stage_aT(st):
        st["aT"] = transpose_feat(st["at"], p_aT, "aT")

    # ---------- Wo + residual + LN2 (next emission; aT copies long done) ----
    def stage_wo_ln2(st):
        aT = st["aT"]
        hr_ts = []
        for c in range(TC):
            pm = ps.tile([128, D], F32, tag="ps", name="wo_ps")
            for k in range(KC):
                nc.tensor.matmul(
                    out=pm,
                    lhsT=aT[:, k * NT + c * 128 : k * NT + (c + 1) * 128],
                    rhs=wo[k],
                    start=(k == 0),
                    stop=(k == KC - 1),
                )
            hr = p_hr.tile([128, D], F32, tag="hr", name="hr")
            nc.vector.tensor_tensor(out=hr, in0=pm, in1=st["h"][c], op=OP.add)
            if flags["bo"]:
                nc.vector.tensor_tensor(out=hr, in0=hr, in1=bo_b, op=OP.add)
            hr_ts.append(hr)

        st["h2"] = [
            layernorm(hr_ts[c], g2_b, be2_b, flags["g2be2"], p_h2, "h2")
            for c in range(TC)
        ]

    # ---------- h2T transposes (next emission, LN2 long done) ----------
    def stage_h2T(st):
        st["h2T"] = transpose_feat(st["h2"], p_h2T, "h2T")

    # ---------- FFN1 (+relu) ----------
    def stage_ffn1(st):
        h2T = st["h2T"]
        rel = p_rel.tile([128, FC * NT], BF16, tag="rel", name="rel")
        nb1 = FC // 4 if BANK2 else FC // 2
        w1grp = 4 if BANK2 else 2
        for b4 in range(nb1):
            if BANK2:
                pm = ps2.tile([128, 4 * NT], F32, tag="ps2", name="f1_ps")
            else:
                pm = ps.tile([128, 2 * NT], F32, tag="ps", name="f1_ps")
            for fh in range(w1grp):
                f = w1grp * b4 + fh
                for k in range(KC):
                    nc.tensor.matmul(
                        out=pm[:, fh * NT : (fh + 1) * NT],
                        lhsT=w1[(k, f)],
                        rhs=h2T[:, k * NT : (k + 1) * NT],
                        start=(k == 0),
                        stop=(k == KC - 1),
                    )
            if flags["b1"]:
                for fh in range(w1grp):
                    f = w1grp * b4 + fh
                    nc.scalar.activation(
                        out=rel[:, f * NT : (f + 1) * NT],
                        in_=pm[:, fh * NT : (fh + 1) * NT],
                        func=AF.Relu,
                        bias=b1c[:, f : f + 1],
                        scale=1.0,
                    )
            elif b4 % 2 == 0:
                nc.scalar.activation(
                    out=rel[:, b4 * w1grp * NT : (b4 + 1) * w1grp * NT],
                    in_=pm,
                    func=AF.Relu,
                    bias=0.0,
                    scale=1.0,
                )
            else:
                nc.vector.tensor_relu(
                    out=rel[:, b4 * w1grp * NT : (b4 + 1) * w1grp * NT], in_=pm
                )
        st["rel"] = rel

    # ---------- FFN2 chunk + residual + store ----------
    def stage_ffn2_chunk(st, it, c):
        row0 = it * NT
        rel = st["rel"]
        pm = ps.tile([128, D], F32, tag="ps", name="f2_ps")
        for f in range(FC):
            nc.tensor.matmul(
                out=pm,
                lhsT=rel[:, f * NT + c * 128 : f * NT + (c + 1) * 128],
                rhs=w2[f],
                start=(f == 0),
                stop=(f == FC - 1),
            )
        o_t = p_out.tile([128, D], F32, tag="o", name="o")
        nc.vector.tensor_tensor(out=o_t, in0=pm, in1=st["h2"][c], op=OP.add)
        if flags["b2"]:
            nc.vector.tensor_tensor(out=o_t, in0=o_t, in1=b2_b, op=OP.add)
        nc.sync.dma_start(
            out=out_dr[row0 + c * 128 : row0 + (c + 1) * 128, :], in_=o_t
        )

    if PIPELINE:
        # 4-deep software pipeline; every PE stage placed so its cross-
        # engine dependencies resolved during earlier PE work:
        #   ln1(j+1) [no PE] | qkv(j) [deps one emission old] |
        #   wo+hr+LN2(j-1) [aT copies aged during qkv] | sc(j,p0) |
        #   h2T(j-2) [LN2(j-2) a full emission old] | sc(j,p1) |
        #   ffn1(j-2) [h2T copy covered by sc p1] | ffn2a(j-2) |
        #   atv(j,p0) atv(j,p1) [exp/mask aged during ffn] |
        #   ffn2b(j-2) [covers at-packs] | aT(j) transposes
        states = {}
        for j in range(n_tiles + 2):
            t_ln, t_at, t_wo, t_ff = j + 1, j, j - 1, j - 2
            if j == 0 and n_tiles > 0:
                states[0] = stage_ln1(0)
            if t_ln < n_tiles:
                states[t_ln] = stage_ln1(t_ln)
            if t_at < n_tiles:
                sta = states[t_at]
                stage_qkv(sta)
            if 0 <= t_wo < n_tiles:
                stage_wo_ln2(states[t_wo])
            if t_at < n_tiles:
                stage_scores_pair(sta, 0)
            if t_ff >= 0:
                stf = states[t_ff]
                stage_h2T(stf)
            if t_at < n_tiles:
                stage_scores_pair(sta, 1)
            if t_ff >= 0:
                stage_ffn1(stf)
                stage_ffn2_chunk(stf, t_ff, 0)
            if t_at < n_tiles:
                stage_atv_pair(sta, 0)
                stage_atv_pair(sta, 1)
            if t_ff >= 0:
                stage_ffn2_chunk(stf, t_ff, 1)
                del states[t_ff]
            if t_at < n_tiles:
                stage_aT(sta)
    else:
        def dummy_out(it):
            row0 = it * NT
            for c in range(TC):
                o_t = p_out.tile([128, D], F32, tag="o", name="o")
                nc.vector.memset(o_t, 0.0)
                nc.sync.dma_start(
                    out=out_dr[row0 + c * 128 : row0 + (c + 1) * 128, :], in_=o_t
                )

        for it in range(n_tiles):
            st = stage_ln1(it)
            if CUT >= 2:
                stage_qkv(st)
            if CUT >= 3:
                stage_scores_pair(st, 0)
                stage_scores_pair(st, 1)
            if CUT >= 4:
                stage_atv_pair(st, 0)
                stage_atv_pair(st, 1)
            if CUT >= 5:
                stage_aT(st)
                stage_wo_ln2(st)
            if CUT >= 6:
                stage_h2T(st)
                stage_ffn1(st)
            if CUT >= 7:
                stage_ffn2_chunk(st, it, 0)
                stage_ffn2_chunk(st, it, 1)
            else:
                dummy_out(it)

    ctx.close()


def prep_inputs(inputs, b_core):
    import ml_dtypes

    f32 = np.float32
    bf16 = ml_dtypes.bfloat16
    wq, wk, wvv = (np.asarray(inputs[k], f32) for k in ("wq", "wk", "wv"))
    bq, bk, bv = (np.asarray(inputs[k], f32) for k in ("bq", "bk", "bv"))
    wo, bo = np.asarray(inputs["wo"], f32), np.asarray(inputs["bo"], f32)
    w1, b1 = np.asarray(inputs["w1"], f32), np.asarray(inputs["b1"], f32)
    w2, b2 = np.asarray(inputs["w2"], f32), np.asarray(inputs["b2"], f32)
    g1, be1 = np.asarray(inputs["g1"], f32), np.asarray(inputs["be1"], f32)
    g2, be2 = np.asarray(inputs["g2"], f32), np.asarray(inputs["be2"], f32)

    # wqk[qi, k, ch] = [128, 128]: cols 0:48 head 2ch, 64:112 head 2ch+1
    wqk = np.zeros((2, KC, 4, 128, 128), f32)
    for qi, w in enumerate((wq, wk)):
        for k in range(KC):
            for ch in range(4):
                wqk[qi, k, ch, :, 0:E] = w[2 * ch][k * 128 : (k + 1) * 128, :]
                wqk[qi, k, ch, :, EP : EP + E] = w[2 * ch + 1][k * 128 : (k + 1) * 128, :]
    bqk = np.zeros((128, 8), f32)
    for qi, b in enumerate((bq, bk)):
        for ch in range(4):
            bqk[0:E, qi * 4 + ch] = b[2 * ch]
            bqk[EP : EP + E, qi * 4 + ch] = b[2 * ch + 1]

    # wv padded: [KC, 128, DP] cols h*64+e
    wv_p = np.zeros((KC, 128, DP), f32)
    for k in range(KC):
        for h in range(H):
            wv_p[k, :, h * EP : h * EP + E] = wvv[h][k * 128 : (k + 1) * 128, :]
    bv_b = np.zeros((DP,), f32)
    for h in range(H):
        bv_b[h * EP : h * EP + E] = bv[h]

    # wo chunks: [KC, 128, D], rows PACKED (h*48) to match attnT layout
    wo_c = np.stack([wo[k * 128 : (k + 1) * 128, :] for k in range(KC)])

    w1_c = np.zeros((KC, FC, 128, 128), f32)
    for k in range(KC):
        for f in range(FC):
            w1_c[k, f] = w1[k * 128 : (k + 1) * 128, f * 128 : (f + 1) * 128]
    b1c = np.zeros((128, FC), f32)
    for f in range(FC):
        b1c[:, f] = b1[f * 128 : (f + 1) * 128]
    w2_c = np.stack([w2[f * 128 : (f + 1) * 128, :] for f in range(FC)])

    # maskT[s, h*128+q]: same batch (s//64==q//64) and causal (s%64 <= q%64)
    s_i = np.arange(128)[:, None]
    q_i = np.arange(128)[None, :]
    base = ((s_i // T) == (q_i // T)) & ((s_i % T) <= (q_i % T))
    maskT = np.tile(base.astype(f32), (1, H))

    bcast = lambda v, w: np.broadcast_to(v[None, :], (128, w)).copy()

    flags = {
        "g1be1": bool(np.any(g1 != 1) or np.any(be1 != 0)),
        "g2be2": bool(np.any(g2 != 1) or np.any(be2 != 0)),
        "bqk": bool(np.any(bq) or np.any(bk)),
        "bv": bool(np.any(bv)),
        "bo": bool(np.any(bo)),
        "b1": bool(np.any(b1)),
        "b2": bool(np.any(b2)),
    }
    common = dict(
        ident=np.eye(128, dtype=bf16),
        maskT=maskT.astype(bf16),
        wqk=wqk.astype(bf16),
        wv=wv_p.astype(bf16),
        wo=wo_c.astype(bf16),
        w1=w1_c.astype(bf16),
        w2=w2_c.astype(bf16),
        bqk=bqk,
        bv_b=bcast(bv_b, DP),
        b1c=b1c,
        g1_b=bcast(g1, D),
        be1_b=bcast(be1, D),
        g2_b=bcast(g2, D),
        be2_b=bcast(be2, D),
        bo_b=bcast(bo, D),
        b2_b=bcast(b2, D),
    )
    return common, flags


CONST_SHAPES = dict(
    ident=(128, 128),
    maskT=(128, H * 128),
    wqk=(2, KC, 4, 128, 128),
    wv=(KC, 128, DP),
    wo=(KC, 128, D),
    w1=(KC, FC, 128, 128),
    w2=(FC, 128, D),
    bqk=(128, 8),
    bv_b=(128, DP),
    b1c=(128, FC),
    g1_b=(128, D),
    be1_b=(128, D),
    g2_b=(128, D),
    be2_b=(128, D),
    bo_b=(128, D),
    b2_b=(128, D),
)


WEIGHT_NAMES = {"ident", "maskT", "wqk", "wv", "wo", "w1", "w2"}


def _make_bacc():
    """Bacc whose act-table placement is steered to the single table that
    serves every activation func this kernel uses (ln, exp, relu, copy,
    identity all live in natural_log_exp_and_others). The default greedy
    choice ping-pongs ln->natural_log / exp->exp_and_others, costing a
    1283ns ACT_TABLE_LOAD 6x per tile. Stripping our funcs from every
    OTHER table (list order, hence act_func_set_id, unchanged) forces the
    pass to pick the shared table once; the emitted id still names a real
    table containing all used funcs, so HW semantics are unchanged."""
    from concourse import bacc
    from concourse.hw_specs import get_activation_tables

    AF = mybir.ActivationFunctionType
    ours = {AF.Ln, AF.Exp, AF.Relu, AF.Copy, AF.Identity}
    target = "natural_log_exp_and_others"

    class _Bacc(bacc.Bacc):
        def insert_act_table_loads(self):
            import bass_rust as _bass_rust

            has_activation = any(
                isinstance(i, mybir.InstActivation)
                for b in self.main_func.blocks
                for i in b.instructions
            )
            if not has_activation:
                return
            tables = [
                (nm, (fs if nm == target else (set(fs) - ours)))
                for nm, fs in get_activation_tables(self.m.arch).items()
            ]
            _bass_rust.insert_act_table_loads(self, tables)

    if not ACT_TRICK:
        return bacc.Bacc("TRN2", target_bir_lowering=False, debug=False)
    return _Bacc("TRN2", target_bir_lowering=False, debug=False)


def build_program(b_core, flags):
    nc = _make_bacc()
    aps = {
        name: nc.dram_tensor(
            name, list(sh), BF16 if name in WEIGHT_NAMES else F32,
            kind="ExternalInput",
        ).ap()
        for name, sh in {**CONST_SHAPES, "x": (b_core, T, D)}.items()
    }
    aps["out"] = nc.dram_tensor("out", [b_core, T, D], F32, kind="ExternalOutput").ap()
    aps["flags"] = flags
    with tile.TileContext(nc) as tc:
        build_body(tc, aps, b_core)
    nc.compile()
    return nc


def kernel(**inputs):
    from concourse.bass_utils import run_bass_kernel_spmd

    x = np.ascontiguousarray(np.asarray(inputs["x"], np.float32))
    common, flags = prep_inputs(inputs, B_CORE)
    nc = build_program(B_CORE, flags)
    in_maps = []
    for c in range(N_CORES):
        m = dict(common)
        m["x"] = np.ascontiguousarray(x[c * B_CORE : (c + 1) * B_CORE])
        in_maps.append(m)
    res = run_bass_kernel_spmd(nc, in_maps, core_ids=list(range(N_CORES)))
    out = np.concatenate([r["out"] for r in res.results], axis=0)
    return out.astype(np.float32)


# revision 5
# speedup vs baseline: 5.1956x; 5.1956x over previous
"""Trainium2 Bass kernel for a dense transformer block (pre-LN, causal MHA + FFN).

Reference computation (per batch element b, T=64 tokens, D=384 features):
    h   = LN(x)*g1 + be1
    q,k,v per-head linears; scores = q k^T / sqrt(48); causal softmax
    attn = probs @ v, concat heads, @ wo + bo
    h    = h + attn              (residual from the *normed* x)
    h2   = LN(h)*g2 + be2
    out  = h2 + relu(h2@w1+b1)@w2 + b2

Sharding: pure data parallel over batch (2048 -> 256 per core, 8 cores),
params replicated; the same single-core program runs SPMD on all 8 cores.

v2 design (vs the f32r baseline):
  - all matmul operands bf16: 1 cyc/row on PE regardless of free size
    (f32r needs free>=256; fp32 is 4 cyc/row), and transposes at 1 cyc/row.
  - attention computes scores TRANSPOSED: scT[s,q] = K Q^T per (batch-pair,
    head) with lhsT=K^T / rhs=Q^T straight from the feature-major QK
    projection - no probs transpose at all.  exp+causal/block mask in
    [s_part, q_free] layout; row sums arrive via a ones-column smuggled into
    V's 64-padding, so attnV emits [q, 48 attn | sum] per head; one
    stride-0-broadcast tensor_tensor both normalizes and packs heads to
    [q, 384].  Per (pair,head): 1 scores mm + 1 attnV mm (vs 4 small mms +
    transpose in the baseline).
  - LN rstd = exp(-0.5*ln(var+eps)): ln+exp+relu+copy live in ONE activation
    table (sqrt does not), killing 2 ACT_TABLE_LOADs (1.3us each) per tile.
  - PSUM->SBUF copies batched per bank; pool/gpsimd engine takes the
    transpose copies; software pipeline runs FFN(i-1) matmuls inside
    attention(i)'s cross-engine stalls to keep PE p-state high.
"""

import os
import sys

sys.path.insert(0, "/opt/trn_rl_repo")

import numpy as np

import concourse.bass as bass
import concourse.tile as tile
from concourse import mybir

# ---- problem constants (hardcoded per contract) ----
B_TOTAL = 2048
T = 64
D = 384
H = 8
E = 48  # head size
EP = 64  # padded head size
F = 4 * D  # ffn hidden 1536
N_CORES = 8
B_CORE = B_TOTAL // N_CORES  # 256
LN_EPS = 1e-5
INV_SQRT_E = float(E) ** -0.5

NB = 4  # batch elems per tile
NT = NB * T  # tokens per tile = 256
KC = D // 128  # 3 contraction chunks for D
FC = F // 128  # 12 chunks for ffn hidden
TC = NT // 128  # 2 token chunks per tile
NP = NB // 2  # batch pairs per tile = 2
DP = H * EP  # padded v width 512

F32 = mybir.dt.float32
BF16 = mybir.dt.bfloat16

PIPELINE = os.environ.get("KPIPE", "1") == "1"
ACT_TRICK = os.environ.get("KACTTRICK", "1") == "1"
LN_ON_ACT = os.environ.get("KLNACT", "1") == "1"
STRIDE0 = os.environ.get("KSTRIDE0", "1") == "1"
BANK2 = os.environ.get("KBANK2", "1") == "1"
MASK_INPLACE = os.environ.get("KMASKIP", "1") == "1"
CUT = float(os.environ.get("KCUT", "7"))  # sequential-mode stage bisect


def build_body(tc, aps, b_core):
    from contextlib import ExitStack

    ctx = ExitStack()
    nc = tc.nc
    n_tiles = b_core * T // NT

    x_dr = aps["x"].rearrange("b t d -> (b t) d")
    out_dr = aps["out"].rearrange("b t d -> (b t) d")

    AF = mybir.ActivationFunctionType
    OP = mybir.AluOpType
    flags = aps["flags"]

    singles = ctx.enter_context(tc.tile_pool(name="singles", bufs=1))

    def load_const(name, shape, src_ap, dt=BF16):
        t_ = singles.tile(list(shape), dt, name=f"sb_{name}")
        nc.sync.dma_start(out=t_, in_=src_ap)
        return t_

    ident = load_const("ident", [128, 128], aps["ident"])
    maskT = load_const("maskT", [128, H * 128], aps["maskT"])
    wqk = {
        (qi, k, ch): load_const(f"wqk{qi}{k}{ch}", [128, 128], aps["wqk"][qi, k, ch])
        for qi in range(2)
        for k in range(KC)
        for ch in range(4)
    }
    wv = {k: load_const(f"wv{k}", [128, DP], aps["wv"][k]) for k in range(KC)}
    wo = {k: load_const(f"wo{k}", [128, D], aps["wo"][k]) for k in range(KC)}
    w1 = {
        (k, f): load_const(f"w1{k}_{f}", [128, 128], aps["w1"][k, f])
        for k in range(KC)
        for f in range(FC)
    }
    w2 = {f: load_const(f"w2{f}", [128, D], aps["w2"][f]) for f in range(FC)}
    bqk = load_const("bqk", [128, 8], aps["bqk"], F32)
    bv_b = load_const("bv_b", [128, DP], aps["bv_b"], F32)
    b1c = load_const("b1c", [128, FC], aps["b1c"], F32)
    g1_b = load_const("g1_b", [128, D], aps["g1_b"], F32)
    be1_b = load_const("be1_b", [128, D], aps["be1_b"], F32)
    g2_b = load_const("g2_b", [128, D], aps["g2_b"], F32)
    be2_b = load_const("be2_b", [128, D], aps["be2_b"], F32)
    bo_b = load_const("bo_b", [128, D], aps["bo_b"], F32)
    b2_b = load_const("b2_b", [128, D], aps["b2_b"], F32)

    eps_t = singles.tile([128, 1], F32, name="eps")
    nc.vector.memset(eps_t, LN_EPS)

    pool = lambda nm, n, **kw: ctx.enter_context(tc.tile_pool(name=nm, bufs=n, **kw))
    ps = pool("ps", 4, space="PSUM")  # 1-bank tiles ring
    ps2 = pool("ps2", 2, space="PSUM")  # 2-bank [128,1024]f32 tiles ring
    # NOTE: pools holding TC=2 chunk tiles per tile-iteration need bufs in
    # CHUNK units: 2 live tiles across the pipeline = 4 chunks + slack.
    p_x = pool("p_x", 4)
    p_h = pool("p_h", 6)
    p_hT = pool("p_hT", 2)
    p_qk = pool("p_qk", 2)  # 2 tags
    p_v = pool("p_v", 2)  # 2 tags
    p_ex = pool("p_ex", 2)  # 2 tags
    p_at = pool("p_at", 2)  # 2 tags
    p_aT = pool("p_aT", 2)
    p_hr = pool("p_hr", 4)
    p_h2 = pool("p_h2", 4)
    p_h2T = pool("p_h2T", 2)
    p_rel = pool("p_rel", 2)
    p_out = pool("p_out", 4)
    p_st = pool("p_st", 6)

    def layernorm(x_t, g_b, be_b, gb_nontriv, pool_, tag):
        """Token-major LN -> bf16. rstd = exp(-0.5*ln(var+eps)) keeps the
        activation table fixed (ln/exp/relu/copy share one table; sqrt does
        not)."""
        st = p_st.tile([128, 6], F32, tag="st", name=f"st_{tag}")
        nc.vector.bn_stats(out=st, in_=x_t)
        mv = p_st.tile([128, 2], F32, tag="mv", name=f"mv_{tag}")
        nc.vector.bn_aggr(out=mv, in_=st)
        mean, var = mv[:, 0:1], mv[:, 1:2]
        lnv = p_st.tile([128, 1], F32, tag="ln", name=f"ln_{tag}")
        nc.scalar.activation(out=lnv, in_=var, func=AF.Ln, bias=eps_t, scale=1.0)
        rstd = p_st.tile([128, 1], F32, tag="rs", name=f"rs_{tag}")
        nc.scalar.activation(out=rstd, in_=lnv, func=AF.Exp, bias=0.0, scale=-0.5)
        nmr = p_st.tile([128, 1], F32, tag="nm", name=f"nm_{tag}")
        nc.vector.scalar_tensor_tensor(
            out=nmr, in0=mean, scalar=-1.0, in1=rstd, op0=OP.mult, op1=OP.mult
        )
        h_t = pool_.tile([128, D], BF16, tag=tag, name=f"h_{tag}")
        if LN_ON_ACT:
            # normalize fused on Act: Identity(rstd*x + nmr), per-partition APs
            nc.scalar.activation(
                out=h_t, in_=x_t, func=AF.Identity, bias=nmr, scale=rstd
            )
        else:
            nc.vector.tensor_scalar(
                out=h_t, in0=x_t, scalar1=rstd, scalar2=nmr, op0=OP.mult, op1=OP.add
            )
        if gb_nontriv:
            nc.vector.tensor_tensor(out=h_t, in0=h_t, in1=g_b, op=OP.mult)
            nc.vector.tensor_tensor(out=h_t, in0=h_t, in1=be_b, op=OP.add)
        return h_t

    def transpose_feat(h_ts, dst_pool, tag):
        """token-major [128, D] x TC (bf16) -> feature-major [128, KC*NT]
        (col k*NT + tok). All TC*KC transposes land in ONE 1-bank bf16 psum
        tile; a single strided DVE copy (bf16 in+out -> 2x mode) moves it."""
        hT = dst_pool.tile([128, KC * NT], BF16, tag=tag, name=f"hT_{tag}")
        hT3 = hT.rearrange("p (k t) -> p k t", k=KC)
        pt_ = ps.tile([128, TC * D], BF16, tag="ps", name=f"tp_{tag}")
        for c in range(TC):
            for k in range(KC):
                nc.tensor.transpose(
                    out=pt_[:, (c * KC + k) * 128 : (c * KC + k) * 128 + 128],
                    in_=h_ts[c][:, k * 128 : (k + 1) * 128],
                    identity=ident,
                )
        src4 = pt_.rearrange("p (c k t) -> p k c t", c=TC, k=KC)
        nc.vector.tensor_copy(out=hT3, in_=src4)
        return hT

    # ---------- stage: load x + LN1 (no PE work; runs a tile ahead) ----------
    def stage_ln1(it):
        row0 = it * NT
        st = {}
        x_ts = []
        for c in range(TC):
            x_t = p_x.tile([128, D], F32, tag="x", name="x")
            nc.sync.dma_start(
                out=x_t, in_=x_dr[row0 + c * 128 : row0 + (c + 1) * 128, :]
            )
            x_ts.append(x_t)
        st["h"] = [
            layernorm(x_ts[c], g1_b, be1_b, flags["g1be1"], p_h, "h") for c in range(TC)
        ]
        return st

    # ---------- stage: hT transposes ----------
    def stage_hT(st):
        st["hT"] = transpose_feat(st["h"], p_hT, "hT")

    # ---------- stage: QKV projections ----------
    def stage_qkv(st):
        hT = st["hT"]

        # Q,K feature-major [128 = 2 heads 64-padded, NT], ch-major cols.
        qk_sb = []
        for qi in range(2):
            sb = p_qk.tile([128, 4 * NT], BF16, tag=f"qk{qi}", name=f"qk{qi}")
            if BANK2:
                pms = [ps2.tile([128, 4 * NT], F32, tag="ps2", name="qk_ps")]
                pmof = [(pms[0], 0)] * 4
            else:
                pms = [ps.tile([128, 2 * NT], F32, tag="ps", name="qk_ps") for _ in range(2)]
                pmof = [(pms[0], 0), (pms[0], 0), (pms[1], 2 * NT), (pms[1], 2 * NT)]
            pm = pms[0]
            for ch in range(4):
                tgt, off = pmof[ch]
                for k in range(KC):
                    nc.tensor.matmul(
                        out=tgt[:, ch * NT - off : (ch + 1) * NT - off],
                        lhsT=wqk[(qi, k, ch)],
                        rhs=hT[:, k * NT : (k + 1) * NT],
                        start=(k == 0),
                        stop=(k == KC - 1),
                    )
            if not BANK2 and not flags["bqk"]:
                for i2, pmx in enumerate(pms):
                    if qi == 0:
                        nc.scalar.copy(out=sb[:, i2 * 2 * NT : (i2 + 1) * 2 * NT], in_=pmx)
                    else:
                        nc.vector.tensor_copy(out=sb[:, i2 * 2 * NT : (i2 + 1) * 2 * NT], in_=pmx)
                qk_sb.append(sb)
                continue
            if flags["bqk"]:
                for ch in range(4):
                    tgt, off = pmof[ch]
                    nc.scalar.activation(
                        out=sb[:, ch * NT : (ch + 1) * NT],
                        in_=tgt[:, ch * NT - off : (ch + 1) * NT - off],
                        func=AF.Identity,
                        bias=bqk[:, qi * 4 + ch : qi * 4 + ch + 1],
                        scale=1.0,
                    )
            elif qi == 0:
                nc.scalar.copy(out=sb, in_=pm)
            else:
                nc.vector.tensor_copy(out=sb, in_=pm)
            lo = p_qk.tile([64, 4 * NT], BF16, tag=f"qlo{qi}", name=f"qlo{qi}")
            nc.sync.dma_start(out=lo[0:48, :], in_=sb[64:112, :])
            qk_sb.append((sb, lo))
        st["qk"] = qk_sb

        # V token-major [128 tok of pair p, DP] with a ones column smuggled
        # at h*64+48 (inside the zero padding) for softmax row sums.
        v_sb = []
        for p in range(NP):
            pm = ps.tile([128, DP], F32, tag="ps", name="v_ps")
            for k in range(KC):
                nc.tensor.matmul(
                    out=pm,
                    lhsT=hT[:, k * NT + p * 128 : k * NT + (p + 1) * 128],
                    rhs=wv[k],
                    start=(k == 0),
                    stop=(k == KC - 1),
                )
            sb = p_v.tile([128, DP], BF16, tag=f"v{p}", name=f"v{p}")
            sb3 = sb.rearrange("p (h e) -> p h e", h=H)
            pm3 = pm.rearrange("p (h e) -> p h e", h=H)
            if flags["bv"]:
                nc.vector.tensor_tensor(out=sb, in0=pm, in1=bv_b, op=OP.add)
            else:
                nc.scalar.copy(out=sb3[:, :, 0:E], in_=pm3[:, :, 0:E])
            nc.gpsimd.memset(sb3[:, :, E], 1.0)
            v_sb.append(sb)
        st["v"] = v_sb
        return st

    # ---------- scores + softmax numerator for one batch pair ----------
    def stage_scores_pair(st, p, phase="all"):
        k_sb, q_sb = st["qk"][1], st["qk"][0]
        if BANK2:
            if phase in ("all", "even"):
                st.setdefault("sc", {})[p] = ps2.tile(
                    [128, H * 128], F32, tag="ps2", name="sc"
                )
            scs_t = st["sc"][p]
            sco = [(scs_t, 0)] * 8
        else:
            sa = ps.tile([128, 4 * 128], F32, tag="ps", name="sca")
            sb2 = ps.tile([128, 4 * 128], F32, tag="ps", name="scb")
            sco = [(sa, 0)] * 4 + [(sb2, 512)] * 4
        heads = range(H)
        if phase == "even":
            heads = range(0, H, 2)
        elif phase == "odd":
            heads = range(1, H, 2)
        for h in heads:
            # even heads live at partitions 0:48 of the projection tile; odd
            # heads were DMA-repacked to partitions 0:48 of the `lo` tile so
            # every matmul runs at tile_position (0,0) - alternating PE
            # partition bases between matmuls wedges the device.
            ksrc = k_sb[h % 2]
            qsrc = q_sb[h % 2]
            ch = h // 2
            sl = slice(ch * NT + p * 128, ch * NT + (p + 1) * 128)
            tgt, o0 = sco[h]
            nc.tensor.matmul(
                out=tgt[:, h * 128 - o0 : (h + 1) * 128 - o0],
                lhsT=ksrc[0:E, sl],
                rhs=qsrc[0:E, sl],
                start=True,
                stop=True,
            )
        if phase == "even":
            return
        ex = p_ex.tile([128, H * 128], BF16, tag=f"ex{p}", name=f"ex{p}")
        if CUT < 2.5:
            # drain psum with a plain copy; no exp/mask
            nc.vector.tensor_copy(out=ex, in_=scs_t if BANK2 else sa)
            st.setdefault("ex", []).append(ex)
            return
        if BANK2:
            nc.scalar.activation(
                out=ex, in_=scs_t, func=AF.Exp, bias=0.0, scale=INV_SQRT_E
            )
        else:
            nc.scalar.activation(
                out=ex[:, 0:512], in_=sa, func=AF.Exp, bias=0.0, scale=INV_SQRT_E
            )
            nc.scalar.activation(
                out=ex[:, 512:1024], in_=sb2, func=AF.Exp, bias=0.0, scale=INV_SQRT_E
            )
        if CUT < 2.9:
            st.setdefault("ex", []).append(ex)
            return
        if MASK_INPLACE:
            nc.vector.tensor_tensor(out=ex, in0=ex, in1=maskT, op=OP.mult)
            st.setdefault("ex", []).append(ex)
        else:
            ex2 = p_ex.tile([128, H * 128], BF16, tag=f"exm{p}", name=f"exm{p}")
            nc.vector.tensor_tensor(out=ex2, in0=ex, in1=maskT, op=OP.mult)
            st.setdefault("ex", []).append(ex2)

    # ---------- attnV + normalize for one batch pair ----------
    def stage_atv_pair(st, p):
        ex, v = st["ex"][p], st["v"][p]
        atp = ps.tile([128, DP], F32, tag="ps", name="at_ps")
        for h in range(H):
            nc.tensor.matmul(
                out=atp[:, h * EP : h * EP + E + 1],
                lhsT=ex[:, h * 128 : (h + 1) * 128],
                rhs=v[:, h * EP : h * EP + E + 1],
                start=True,
                stop=True,
            )
        atp3 = atp.rearrange("p (h e) -> p h e", h=H)
        rr = p_st.tile([128, H], F32, tag="rr", name="rr")
        nc.vector.reciprocal(out=rr, in_=atp3[:, :, E])
        sb = p_at.tile([128, H * E], BF16, tag=f"at{p}", name=f"at{p}")
        if STRIDE0:
            nc.vector.tensor_tensor(
                out=sb.rearrange("p (h e) -> p h e", h=H),
                in0=atp3[:, :, 0:E],
                in1=rr.broadcast_to([128, H, E]),
                op=OP.mult,
            )
        else:
            for h in range(H):
                nc.vector.tensor_scalar(
                    out=sb[:, h * E : (h + 1) * E],
                    in0=atp[:, h * EP : h * EP + E],
                    scalar1=rr[:, h : h + 1],
                    scalar2=None,
                    op0=OP.mult,
                )
        st.setdefault("at", []).append(sb)

    # ---------- attnT transposes (end of the tile's emission) ----------
    def stage_aT(st):
        st["aT"] = transpose_feat(st["at"], p_aT, "aT")

    # ---------- Wo + residual + LN2 (next emission; aT copies long done) ----
    def stage_wo_ln2(st):
        aT = st["aT"]
        hr_ts = []
        for c in range(TC):
            pm = ps.tile([128, D], F32, tag="ps", name="wo_ps")
            for k in range(KC):
                nc.tensor.matmul(
                    out=pm,
                    lhsT=aT[:, k * NT + c * 128 : k * NT + (c + 1) * 128],
                    rhs=wo[k],
                    start=(k == 0),
                    stop=(k == KC - 1),
                )
            hr = p_hr.tile([128, D], F32, tag="hr", name="hr")
            nc.vector.tensor_tensor(out=hr, in0=pm, in1=st["h"][c], op=OP.add)
            if flags["bo"]:
                nc.vector.tensor_tensor(out=hr, in0=hr, in1=bo_b, op=OP.add)
            hr_ts.append(hr)

        st["h2"] = [
            layernorm(hr_ts[c], g2_b, be2_b, flags["g2be2"], p_h2, "h2")
            for c in range(TC)
        ]

    # ---------- h2T transposes (next emission, LN2 long done) ----------
    def stage_h2T(st):
        st["h2T"] = transpose_feat(st["h2"], p_h2T, "h2T")

    # ---------- FFN1 (+relu) ----------
    def stage_ffn1(st):
        h2T = st["h2T"]
        rel = p_rel.tile([128, FC * NT], BF16, tag="rel", name="rel")
        nb1 = FC // 4 if BANK2 else FC // 2
        w1grp = 4 if BANK2 else 2
        for b4 in range(nb1):
            if BANK2:
                pm = ps2.tile([128, 4 * NT], F32, tag="ps2", name="f1_ps")
            else:
                pm = ps.tile([128, 2 * NT], F32, tag="ps", name="f1_ps")
            for fh in range(w1grp):
                f = w1grp * b4 + fh
                for k in range(KC):
                    nc.tensor.matmul(
                        out=pm[:, fh * NT : (fh + 1) * NT],
                        lhsT=w1[(k, f)],
                        rhs=h2T[:, k * NT : (k + 1) * NT],
                        start=(k == 0),
                        stop=(k == KC - 1),
                    )
            if flags["b1"]:
                for fh in range(w1grp):
                    f = w1grp * b4 + fh
                    nc.scalar.activation(
                        out=rel[:, f * NT : (f + 1) * NT],
                        in_=pm[:, fh * NT : (fh + 1) * NT],
                        func=AF.Relu,
                        bias=b1c[:, f : f + 1],
                        scale=1.0,
                    )
            elif b4 % 2 == 0:
                nc.scalar.activation(
                    out=rel[:, b4 * w1grp * NT : (b4 + 1) * w1grp * NT],
                    in_=pm,
                    func=AF.Relu,
                    bias=0.0,
                    scale=1.0,
                )
            else:
                nc.vector.tensor_relu(
                    out=rel[:, b4 * w1grp * NT : (b4 + 1) * w1grp * NT], in_=pm
                )
        st["rel"] = rel

    # ---------- FFN2 chunk + residual + store ----------
    def stage_ffn2_chunk(st, it, c):
        row0 = it * NT
        rel = st["rel"]
        pm = ps.tile([128, D], F32, tag="ps", name="f2_ps")
        for f in range(FC):
            nc.tensor.matmul(
                out=pm,
                lhsT=rel[:, f * NT + c * 128 : f * NT + (c + 1) * 128],
                rhs=w2[f],
                start=(f == 0),
                stop=(f == FC - 1),
            )
        o_t = p_out.tile([128, D], F32, tag="o", name="o")
        nc.vector.tensor_tensor(out=o_t, in0=pm, in1=st["h2"][c], op=OP.add)
        if flags["b2"]:
            nc.vector.tensor_tensor(out=o_t, in0=o_t, in1=b2_b, op=OP.add)
        nc.sync.dma_start(
            out=out_dr[row0 + c * 128 : row0 + (c + 1) * 128, :], in_=o_t
        )

    if PIPELINE:
        # 4-deep software pipeline; every PE stage placed so its cross-
        # engine dependencies resolved during earlier PE work:
        #   ln1(j+1) [no PE] | qkv(j) [deps one emission old] |
        #   wo+hr+LN2(j-1) [aT copies aged during qkv] | sc(j,p0) |
        #   h2T(j-2) [LN2(j-2) a full emission old] | sc(j,p1) |
        #   ffn1(j-2) [h2T copy covered by sc p1] | ffn2a(j-2) |
        #   atv(j,p0) atv(j,p1) [exp/mask aged during ffn] |
        #   ffn2b(j-2) [covers at-packs] | aT(j) transposes
        states = {}
        for j in range(n_tiles + 2):
            t_ln, t_at, t_wo, t_ff = j + 1, j, j - 1, j - 2
            if j == 0 and n_tiles > 0:
                states[0] = stage_ln1(0)
            if t_ln < n_tiles:
                states[t_ln] = stage_ln1(t_ln)
            if t_at < n_tiles:
                sta = states[t_at]
                stage_hT(sta)
            if 0 <= t_wo < n_tiles:
                stage_wo_ln2(states[t_wo])
            if t_at < n_tiles:
                stage_qkv(sta)
                stage_scores_pair(sta, 0, "even")
                stage_scores_pair(sta, 1, "even")
            if t_ff >= 0:
                stf = states[t_ff]
                stage_h2T(stf)
            if t_at < n_tiles:
                stage_scores_pair(sta, 0, "odd")
                stage_scores_pair(sta, 1, "odd")
            if t_ff >= 0:
                stage_ffn1(stf)
                stage_ffn2_chunk(stf, t_ff, 0)
            if t_at < n_tiles:
                stage_atv_pair(sta, 0)
                stage_atv_pair(sta, 1)
            if t_ff >= 0:
                stage_ffn2_chunk(stf, t_ff, 1)
                del states[t_ff]
            if t_at < n_tiles:
                stage_aT(sta)
    else:
        def dummy_out(it):
            row0 = it * NT
            for c in range(TC):
                o_t = p_out.tile([128, D], F32, tag="o", name="o")
                nc.vector.memset(o_t, 0.0)
                nc.sync.dma_start(
                    out=out_dr[row0 + c * 128 : row0 + (c + 1) * 128, :], in_=o_t
                )

        for it in range(n_tiles):
            st = stage_ln1(it)
            if CUT >= 2:
                stage_hT(st)
                stage_qkv(st)
            if CUT >= 3:
                stage_scores_pair(st, 0)
                stage_scores_pair(st, 1)
            if CUT >= 4:
                stage_atv_pair(st, 0)
                stage_atv_pair(st, 1)
            if CUT >= 5:
                stage_aT(st)
                stage_wo_ln2(st)
            if CUT >= 6:
                stage_h2T(st)
                stage_ffn1(st)
            if CUT >= 7:
                stage_ffn2_chunk(st, it, 0)
                stage_ffn2_chunk(st, it, 1)
            else:
                dummy_out(it)

    ctx.close()


def prep_inputs(inputs, b_core):
    import ml_dtypes

    f32 = np.float32
    bf16 = ml_dtypes.bfloat16
    wq, wk, wvv = (np.asarray(inputs[k], f32) for k in ("wq", "wk", "wv"))
    bq, bk, bv = (np.asarray(inputs[k], f32) for k in ("bq", "bk", "bv"))
    wo, bo = np.asarray(inputs["wo"], f32), np.asarray(inputs["bo"], f32)
    w1, b1 = np.asarray(inputs["w1"], f32), np.asarray(inputs["b1"], f32)
    w2, b2 = np.asarray(inputs["w2"], f32), np.asarray(inputs["b2"], f32)
    g1, be1 = np.asarray(inputs["g1"], f32), np.asarray(inputs["be1"], f32)
    g2, be2 = np.asarray(inputs["g2"], f32), np.asarray(inputs["be2"], f32)

    # wqk[qi, k, ch] = [128, 128]: cols 0:48 head 2ch, 64:112 head 2ch+1
    wqk = np.zeros((2, KC, 4, 128, 128), f32)
    for qi, w in enumerate((wq, wk)):
        for k in range(KC):
            for ch in range(4):
                wqk[qi, k, ch, :, 0:E] = w[2 * ch][k * 128 : (k + 1) * 128, :]
                wqk[qi, k, ch, :, EP : EP + E] = w[2 * ch + 1][k * 128 : (k + 1) * 128, :]
    bqk = np.zeros((128, 8), f32)
    for qi, b in enumerate((bq, bk)):
        for ch in range(4):
            bqk[0:E, qi * 4 + ch] = b[2 * ch]
            bqk[EP : EP + E, qi * 4 + ch] = b[2 * ch + 1]

    # wv padded: [KC, 128, DP] cols h*64+e
    wv_p = np.zeros((KC, 128, DP), f32)
    for k in range(KC):
        for h in range(H):
            wv_p[k, :, h * EP : h * EP + E] = wvv[h][k * 128 : (k + 1) * 128, :]
    bv_b = np.zeros((DP,), f32)
    for h in range(H):
        bv_b[h * EP : h * EP + E] = bv[h]

    # wo chunks: [KC, 128, D], rows PACKED (h*48) to match attnT layout
    wo_c = np.stack([wo[k * 128 : (k + 1) * 128, :] for k in range(KC)])

    w1_c = np.zeros((KC, FC, 128, 128), f32)
    for k in range(KC):
        for f in range(FC):
            w1_c[k, f] = w1[k * 128 : (k + 1) * 128, f * 128 : (f + 1) * 128]
    b1c = np.zeros((128, FC), f32)
    for f in range(FC):
        b1c[:, f] = b1[f * 128 : (f + 1) * 128]
    w2_c = np.stack([w2[f * 128 : (f + 1) * 128, :] for f in range(FC)])

    # maskT[s, h*128+q]: same batch (s//64==q//64) and causal (s%64 <= q%64)
    s_i = np.arange(128)[:, None]
    q_i = np.arange(128)[None, :]
    base = ((s_i // T) == (q_i // T)) & ((s_i % T) <= (q_i % T))
    maskT = np.tile(base.astype(f32), (1, H))

    bcast = lambda v, w: np.broadcast_to(v[None, :], (128, w)).copy()

    flags = {
        "g1be1": bool(np.any(g1 != 1) or np.any(be1 != 0)),
        "g2be2": bool(np.any(g2 != 1) or np.any(be2 != 0)),
        "bqk": bool(np.any(bq) or np.any(bk)),
        "bv": bool(np.any(bv)),
        "bo": bool(np.any(bo)),
        "b1": bool(np.any(b1)),
        "b2": bool(np.any(b2)),
    }
    common = dict(
        ident=np.eye(128, dtype=bf16),
        maskT=maskT.astype(bf16),
        wqk=wqk.astype(bf16),
        wv=wv_p.astype(bf16),
        wo=wo_c.astype(bf16),
        w1=w1_c.astype(bf16),
        w2=w2_c.astype(bf16),
        bqk=bqk,
        bv_b=bcast(bv_b, DP),
        b1c=b1c,
        g1_b=bcast(g1, D),
        be1_b=bcast(be1, D),
        g2_b=bcast(g2, D),
        be2_b=bcast(be2, D),
        bo_b=bcast(bo, D),
        b2_b=bcast(b2, D),
    )
    return common, flags


CONST_SHAPES = dict(
    ident=(128, 128),
    maskT=(128, H * 128),
    wqk=(2, KC, 4, 128, 128),
    wv=(KC, 128, DP),
    wo=(KC, 128, D),
    w1=(KC, FC, 128, 128),
    w2=(FC, 128, D),
    bqk=(128, 8),
    bv_b=(128, DP),
    b1c=(128, FC),
    g1_b=(128, D),
    be1_b=(128, D),
    g2_b=(128, D),
    be2_b=(128, D),
    bo_b=(128, D),
    b2_b=(128, D),
)


WEIGHT_NAMES = {"ident", "maskT", "wqk", "wv", "wo", "w1", "w2"}


def _make_bacc():
    """Bacc whose act-table placement is steered to the single table that
    serves every activation func this kernel uses (ln, exp, relu, copy,
    identity all live in natural_log_exp_and_others). The default greedy
    choice ping-pongs ln->natural_log / exp->exp_and_others, costing a
    1283ns ACT_TABLE_LOAD 6x per tile. Stripping our funcs from every
    OTHER table (list order, hence act_func_set_id, unchanged) forces the
    pass to pick the shared table once; the emitted id still names a real
    table containing all used funcs, so HW semantics are unchanged."""
    from concourse import bacc
    from concourse.hw_specs import get_activation_tables

    AF = mybir.ActivationFunctionType
    ours = {AF.Ln, AF.Exp, AF.Relu, AF.Copy, AF.Identity}
    target = "natural_log_exp_and_others"

    class _Bacc(bacc.Bacc):
        def insert_act_table_loads(self):
            import bass_rust as _bass_rust

            has_activation = any(
                isinstance(i, mybir.InstActivation)
                for b in self.main_func.blocks
                for i in b.instructions
            )
            if not has_activation:
                return
            tables = [
                (nm, (fs if nm == target else (set(fs) - ours)))
                for nm, fs in get_activation_tables(self.m.arch).items()
            ]
            _bass_rust.insert_act_table_loads(self, tables)

    if not ACT_TRICK:
        return bacc.Bacc("TRN2", target_bir_lowering=False, debug=False)
    return _Bacc("TRN2", target_bir_lowering=False, debug=False)


def build_program(b_core, flags):
    nc = _make_bacc()
    aps = {
        name: nc.dram_tensor(
            name, list(sh), BF16 if name in WEIGHT_NAMES else F32,
            kind="ExternalInput",
        ).ap()
        for name, sh in {**CONST_SHAPES, "x": (b_core, T, D)}.items()
    }
    aps["out"] = nc.dram_tensor("out", [b_core, T, D], F32, kind="ExternalOutput").ap()
    aps["flags"] = flags
    with tile.TileContext(nc) as tc:
        build_body(tc, aps, b_core)
    nc.compile()
    return nc


def kernel(**inputs):
    from concourse.bass_utils import run_bass_kernel_spmd

    x = np.ascontiguousarray(np.asarray(inputs["x"], np.float32))
    common, flags = prep_inputs(inputs, B_CORE)
    nc = build_program(B_CORE, flags)
    in_maps = []
    for c in range(N_CORES):
        m = dict(common)
        m["x"] = np.ascontiguousarray(x[c * B_CORE : (c + 1) * B_CORE])
        in_maps.append(m)
    res = run_bass_kernel_spmd(nc, in_maps, core_ids=list(range(N_CORES)))
    out = np.concatenate([r["out"] for r in res.results], axis=0)
    return out.astype(np.float32)


# revision 6
# speedup vs baseline: 5.4519x; 1.0493x over previous
"""Trainium2 Bass kernel for a dense transformer block (pre-LN, causal MHA + FFN).

Reference computation (per batch element b, T=64 tokens, D=384 features):
    h   = LN(x)*g1 + be1
    q,k,v per-head linears; scores = q k^T / sqrt(48); causal softmax
    attn = probs @ v, concat heads, @ wo + bo
    h    = h + attn              (residual from the *normed* x)
    h2   = LN(h)*g2 + be2
    out  = h2 + relu(h2@w1+b1)@w2 + b2

Sharding: pure data parallel over batch (2048 -> 256 per core, 8 cores),
params replicated; the same single-core program runs SPMD on all 8 cores.

v2 design (vs the f32r baseline):
  - all matmul operands bf16: 1 cyc/row on PE regardless of free size
    (f32r needs free>=256; fp32 is 4 cyc/row), and transposes at 1 cyc/row.
  - attention computes scores TRANSPOSED: scT[s,q] = K Q^T per (batch-pair,
    head) with lhsT=K^T / rhs=Q^T straight from the feature-major QK
    projection - no probs transpose at all.  exp+causal/block mask in
    [s_part, q_free] layout; row sums arrive via a ones-column smuggled into
    V's 64-padding, so attnV emits [q, 48 attn | sum] per head; one
    stride-0-broadcast tensor_tensor both normalizes and packs heads to
    [q, 384].  Per (pair,head): 1 scores mm + 1 attnV mm (vs 4 small mms +
    transpose in the baseline).
  - LN rstd = exp(-0.5*ln(var+eps)): ln+exp+relu+copy live in ONE activation
    table (sqrt does not), killing 2 ACT_TABLE_LOADs (1.3us each) per tile.
  - PSUM->SBUF copies batched per bank; pool/gpsimd engine takes the
    transpose copies; software pipeline runs FFN(i-1) matmuls inside
    attention(i)'s cross-engine stalls to keep PE p-state high.
"""

import os
import sys

sys.path.insert(0, "/opt/trn_rl_repo")

import numpy as np

import concourse.bass as bass
import concourse.tile as tile
from concourse import mybir

# ---- problem constants (hardcoded per contract) ----
B_TOTAL = 2048
T = 64
D = 384
H = 8
E = 48  # head size
EP = 64  # padded head size
F = 4 * D  # ffn hidden 1536
N_CORES = 8
B_CORE = B_TOTAL // N_CORES  # 256
LN_EPS = 1e-5
INV_SQRT_E = float(E) ** -0.5

NB = 4  # batch elems per tile
NT = NB * T  # tokens per tile = 256
KC = D // 128  # 3 contraction chunks for D
FC = F // 128  # 12 chunks for ffn hidden
TC = NT // 128  # 2 token chunks per tile
NP = NB // 2  # batch pairs per tile = 2
DP = H * (E + 1)  # packed v width 392 (48 data + ones col per head)

F32 = mybir.dt.float32
BF16 = mybir.dt.bfloat16

PIPELINE = os.environ.get("KPIPE", "1") == "1"
ACT_TRICK = os.environ.get("KACTTRICK", "1") == "1"
LN_ON_ACT = os.environ.get("KLNACT", "1") == "1"
STRIDE0 = os.environ.get("KSTRIDE0", "1") == "1"
BANK2 = os.environ.get("KBANK2", "1") == "1"
MASK_INPLACE = os.environ.get("KMASKIP", "1") == "1"
CUT = float(os.environ.get("KCUT", "7"))  # sequential-mode stage bisect


def build_body(tc, aps, b_core):
    from contextlib import ExitStack

    ctx = ExitStack()
    nc = tc.nc
    n_tiles = b_core * T // NT

    x_dr = aps["x"].rearrange("b t d -> (b t) d")
    out_dr = aps["out"].rearrange("b t d -> (b t) d")

    AF = mybir.ActivationFunctionType
    OP = mybir.AluOpType
    flags = aps["flags"]

    singles = ctx.enter_context(tc.tile_pool(name="singles", bufs=1))
    prefetched_x = {}

    def load_const(name, shape, src_ap, dt=BF16):
        t_ = singles.tile(list(shape), dt, name=f"sb_{name}")
        nc.sync.dma_start(out=t_, in_=src_ap)
        return t_

    # x for the first two tiles is DMA'd BEFORE the ~80 const loads so the
    # pipeline prologue isn't serialized behind them in the DMA queue.
    x_pool_early = ctx.enter_context(tc.tile_pool(name="p_x_early", bufs=1))
    for it0 in range(min(2, n_tiles)):
        row0 = it0 * NT
        xts = []
        for c in range(TC):
            x_t = x_pool_early.tile([128, D], F32, name=f"xe{it0}_{c}")
            nc.sync.dma_start(
                out=x_t, in_=x_dr[row0 + c * 128 : row0 + (c + 1) * 128, :]
            )
            xts.append(x_t)
        prefetched_x[it0] = xts

    ident = load_const("ident", [128, 128], aps["ident"])
    maskT = load_const("maskT", [128, H * 128], aps["maskT"])
    wqk = {
        (k, ch): load_const(f"wqk1{k}{ch}", [128, 128], aps["wqk"][1, k, ch])
        for k in range(KC)
        for ch in range(4)
    }
    wqz = {
        (h, k): load_const(f"wqz{h}_{k}", [128, 128], aps["wqz"][h, k])
        for h in range(H)
        for k in range(KC)
    }
    wv = {k: load_const(f"wv{k}", [128, DP], aps["wv"][k]) for k in range(KC)}
    wo = {k: load_const(f"wo{k}", [128, D], aps["wo"][k]) for k in range(KC)}
    w1 = {
        (k, f): load_const(f"w1{k}_{f}", [128, 128], aps["w1"][k, f])
        for k in range(KC)
        for f in range(FC)
    }
    w2 = {f: load_const(f"w2{f}", [128, D], aps["w2"][f]) for f in range(FC)}
    bqk = load_const("bqk", [128, 12], aps["bqk"], F32)
    bv_b = load_const("bv_b", [128, DP], aps["bv_b"], F32)
    b1c = load_const("b1c", [128, FC], aps["b1c"], F32)
    g1_b = load_const("g1_b", [128, D], aps["g1_b"], F32)
    be1_b = load_const("be1_b", [128, D], aps["be1_b"], F32)
    g2_b = load_const("g2_b", [128, D], aps["g2_b"], F32)
    be2_b = load_const("be2_b", [128, D], aps["be2_b"], F32)
    bo_b = load_const("bo_b", [128, D], aps["bo_b"], F32)
    b2_b = load_const("b2_b", [128, D], aps["b2_b"], F32)

    eps_t = singles.tile([128, 1], F32, name="eps")
    nc.vector.memset(eps_t, LN_EPS)

    pool = lambda nm, n, **kw: ctx.enter_context(tc.tile_pool(name=nm, bufs=n, **kw))
    ps = pool("ps", 4, space="PSUM")  # 1-bank tiles ring
    ps2 = pool("ps2", 2, space="PSUM")  # 2-bank [128,1024]f32 tiles ring
    # NOTE: pools holding TC=2 chunk tiles per tile-iteration need bufs in
    # CHUNK units: 2 live tiles across the pipeline = 4 chunks + slack.
    p_x = pool("p_x", 4)
    p_h = pool("p_h", 6)
    p_hT = pool("p_hT", 2)
    p_qk = pool("p_qk", 2)  # 2 tags
    p_v = pool("p_v", 2)  # 2 tags
    p_ex = pool("p_ex", 2)  # 2 tags
    p_at = pool("p_at", 2)  # 2 tags
    p_aT = pool("p_aT", 2)
    p_hr = pool("p_hr", 4)
    p_h2 = pool("p_h2", 4)
    p_h2T = pool("p_h2T", 2)
    p_rel = pool("p_rel", 2)
    p_out = pool("p_out", 4)
    p_st = pool("p_st", 6)

    def layernorm(x_t, g_b, be_b, gb_nontriv, pool_, tag):
        """Token-major LN -> bf16. rstd = exp(-0.5*ln(var+eps)) keeps the
        activation table fixed (ln/exp/relu/copy share one table; sqrt does
        not)."""
        st = p_st.tile([128, 6], F32, tag="st", name=f"st_{tag}")
        nc.vector.bn_stats(out=st, in_=x_t)
        mv = p_st.tile([128, 2], F32, tag="mv", name=f"mv_{tag}")
        nc.vector.bn_aggr(out=mv, in_=st)
        mean, var = mv[:, 0:1], mv[:, 1:2]
        lnv = p_st.tile([128, 1], F32, tag="ln", name=f"ln_{tag}")
        nc.scalar.activation(out=lnv, in_=var, func=AF.Ln, bias=eps_t, scale=1.0)
        rstd = p_st.tile([128, 1], F32, tag="rs", name=f"rs_{tag}")
        nc.scalar.activation(out=rstd, in_=lnv, func=AF.Exp, bias=0.0, scale=-0.5)
        nmr = p_st.tile([128, 1], F32, tag="nm", name=f"nm_{tag}")
        nc.vector.scalar_tensor_tensor(
            out=nmr, in0=mean, scalar=-1.0, in1=rstd, op0=OP.mult, op1=OP.mult
        )
        h_t = pool_.tile([128, D], BF16, tag=tag, name=f"h_{tag}")
        if LN_ON_ACT:
            # normalize fused on Act: Identity(rstd*x + nmr), per-partition APs
            nc.scalar.activation(
                out=h_t, in_=x_t, func=AF.Identity, bias=nmr, scale=rstd
            )
        else:
            nc.vector.tensor_scalar(
                out=h_t, in0=x_t, scalar1=rstd, scalar2=nmr, op0=OP.mult, op1=OP.add
            )
        if gb_nontriv:
            nc.vector.tensor_tensor(out=h_t, in0=h_t, in1=g_b, op=OP.mult)
            nc.vector.tensor_tensor(out=h_t, in0=h_t, in1=be_b, op=OP.add)
        return h_t

    def transpose_feat(h_ts, dst_pool, tag):
        """token-major [128, D] x TC (bf16) -> feature-major [128, KC*NT]
        (col k*NT + tok). All TC*KC transposes land in ONE 1-bank bf16 psum
        tile; a single strided DVE copy (bf16 in+out -> 2x mode) moves it."""
        hT = dst_pool.tile([128, KC * NT], BF16, tag=tag, name=f"hT_{tag}")
        hT3 = hT.rearrange("p (k t) -> p k t", k=KC)
        pt_ = ps.tile([128, TC * D], BF16, tag="ps", name=f"tp_{tag}")
        for c in range(TC):
            for k in range(KC):
                nc.tensor.transpose(
                    out=pt_[:, (c * KC + k) * 128 : (c * KC + k) * 128 + 128],
                    in_=h_ts[c][:, k * 128 : (k + 1) * 128],
                    identity=ident,
                )
        src4 = pt_.rearrange("p (c k t) -> p k c t", c=TC, k=KC)
        nc.vector.tensor_copy(out=hT3, in_=src4)
        return hT

    # ---------- stage: load x + LN1 (no PE work; runs a tile ahead) ----------
    def stage_ln1(it):
        row0 = it * NT
        st = {}
        if it in prefetched_x:
            x_ts = prefetched_x[it]
        else:
            x_ts = []
            for c in range(TC):
                x_t = p_x.tile([128, D], F32, tag="x", name="x")
                nc.sync.dma_start(
                    out=x_t, in_=x_dr[row0 + c * 128 : row0 + (c + 1) * 128, :]
                )
                x_ts.append(x_t)
        st["h"] = [
            layernorm(x_ts[c], g1_b, be1_b, flags["g1be1"], p_h, "h") for c in range(TC)
        ]
        return st

    # ---------- stage: hT transposes ----------
    def stage_hT(st):
        st["hT"] = transpose_feat(st["h"], p_hT, "hT")

    # ---------- stage: QKV projections ----------
    def stage_qkv(st):
        hT = st["hT"]

        # Q,K feature-major [128 = 2 heads 64-padded, NT], ch-major cols.
        # K: packed 2-heads-per-128 layout (lhsT side of scoresT; full-128
        # contraction slices, base 0)
        k_sb = p_qk.tile([128, 4 * NT], BF16, tag="ksb", name="ksb")
        pm = ps2.tile([128, 4 * NT], F32, tag="ps2", name="qk_ps")
        for ch in range(4):
            for k in range(KC):
                nc.tensor.matmul(
                    out=pm[:, ch * NT : (ch + 1) * NT],
                    lhsT=wqk[(k, ch)],
                    rhs=hT[:, k * NT : (k + 1) * NT],
                    start=(k == 0),
                    stop=(k == KC - 1),
                )
        if flags["bqk"]:
            for ch in range(4):
                nc.scalar.activation(
                    out=k_sb[:, ch * NT : (ch + 1) * NT],
                    in_=pm[:, ch * NT : (ch + 1) * NT],
                    func=AF.Identity,
                    bias=bqk[:, 8 + ch : 8 + ch + 1],
                    scale=1.0,
                )
        else:
            nc.vector.tensor_copy(out=k_sb, in_=pm)

        # Q: one 256-col block per head, rows zero outside the head's 48
        # dims (wqz zero-padding), so the scoresT matmul can contract all
        # 128 partitions -- the partner head's K rows are annihilated by
        # Q-side zeros, every matmul sits at partition base 0.
        q_sb = p_qk.tile([128, H * NT], BF16, tag="qsb", name="qsb")
        for g in range(2):
            pmq = ps2.tile([128, 4 * NT], F32, tag="ps2", name="qz_ps")
            for hh in range(4):
                h = 4 * g + hh
                for k in range(KC):
                    nc.tensor.matmul(
                        out=pmq[:, hh * NT : (hh + 1) * NT],
                        lhsT=wqz[(h, k)],
                        rhs=hT[:, k * NT : (k + 1) * NT],
                        start=(k == 0),
                        stop=(k == KC - 1),
                    )
            if flags["bqk"]:
                for hh in range(4):
                    h = 4 * g + hh
                    nc.scalar.activation(
                        out=q_sb[:, h * NT : (h + 1) * NT],
                        in_=pmq[:, hh * NT : (hh + 1) * NT],
                        func=AF.Identity,
                        bias=bqk[:, h : h + 1],
                        scale=1.0,
                    )
            else:
                nc.scalar.copy(
                    out=q_sb[:, g * 4 * NT : (g + 1) * 4 * NT], in_=pmq
                )
        st["qk"] = (q_sb, k_sb)

        # V token-major [128 tok of pair p, DP] with a ones column smuggled
        # at h*64+48 (inside the zero padding) for softmax row sums.
        v_sb = []
        for p in range(NP):
            pm = ps.tile([128, DP], F32, tag="ps", name="v_ps")
            for k in range(KC):
                nc.tensor.matmul(
                    out=pm,
                    lhsT=hT[:, k * NT + p * 128 : k * NT + (p + 1) * 128],
                    rhs=wv[k],
                    start=(k == 0),
                    stop=(k == KC - 1),
                )
            sb = p_v.tile([128, DP], BF16, tag=f"v{p}", name=f"v{p}")
            sb3 = sb.rearrange("p (h e) -> p h e", h=H)
            pm3 = pm.rearrange("p (h e) -> p h e", h=H)
            if flags["bv"]:
                nc.vector.tensor_tensor(out=sb, in0=pm, in1=bv_b, op=OP.add)
            else:
                nc.scalar.copy(out=sb3[:, :, 0:E], in_=pm3[:, :, 0:E])
            nc.gpsimd.memset(sb3[:, :, E], 1.0)
            v_sb.append(sb)
        st["v"] = v_sb
        return st

    # ---------- scores + softmax numerator for one batch pair ----------
    def stage_scores_pair(st, p, phase="all"):
        q_sb, k_sb = st["qk"]
        if BANK2:
            if phase in ("all", "even"):
                st.setdefault("sc", {})[p] = ps2.tile(
                    [128, H * 128], F32, tag="ps2", name="sc"
                )
            scs_t = st["sc"][p]
            sco = [(scs_t, 0)] * 8
        else:
            sa = ps.tile([128, 4 * 128], F32, tag="ps", name="sca")
            sb2 = ps.tile([128, 4 * 128], F32, tag="ps", name="scb")
            sco = [(sa, 0)] * 4 + [(sb2, 512)] * 4
        heads = range(H)
        if phase == "even":
            heads = range(0, H, 2)
        elif phase == "odd":
            heads = range(1, H, 2)
        for h in heads:
            ch = h // 2
            sl = slice(ch * NT + p * 128, ch * NT + (p + 1) * 128)
            tgt, o0 = sco[h]
            nc.tensor.matmul(
                out=tgt[:, h * 128 - o0 : (h + 1) * 128 - o0],
                lhsT=k_sb[:, sl],
                rhs=q_sb[:, h * NT + p * 128 : h * NT + (p + 1) * 128],
                start=True,
                stop=True,
            )
        if phase == "even":
            return
        ex = p_ex.tile([128, H * 128], BF16, tag=f"ex{p}", name=f"ex{p}")
        if CUT < 2.5:
            # drain psum with a plain copy; no exp/mask
            nc.vector.tensor_copy(out=ex, in_=scs_t if BANK2 else sa)
            st.setdefault("ex", []).append(ex)
            return
        if BANK2:
            nc.scalar.activation(
                out=ex, in_=scs_t, func=AF.Exp, bias=0.0, scale=INV_SQRT_E
            )
        else:
            nc.scalar.activation(
                out=ex[:, 0:512], in_=sa, func=AF.Exp, bias=0.0, scale=INV_SQRT_E
            )
            nc.scalar.activation(
                out=ex[:, 512:1024], in_=sb2, func=AF.Exp, bias=0.0, scale=INV_SQRT_E
            )
        if CUT < 2.9:
            st.setdefault("ex", []).append(ex)
            return
        if MASK_INPLACE:
            nc.vector.tensor_tensor(out=ex, in0=ex, in1=maskT, op=OP.mult)
            st.setdefault("ex", []).append(ex)
        else:
            ex2 = p_ex.tile([128, H * 128], BF16, tag=f"exm{p}", name=f"exm{p}")
            nc.vector.tensor_tensor(out=ex2, in0=ex, in1=maskT, op=OP.mult)
            st.setdefault("ex", []).append(ex2)

    # ---------- attnV + normalize for one batch pair ----------
    def stage_atv_pair(st, p):
        ex, v = st["ex"][p], st["v"][p]
        atp = ps.tile([128, H * EP], F32, tag="ps", name="at_ps")
        for h in range(H):
            nc.tensor.matmul(
                out=atp[:, h * EP : h * EP + E + 1],
                lhsT=ex[:, h * 128 : (h + 1) * 128],
                rhs=v[:, h * (E + 1) : (h + 1) * (E + 1)],
                start=True,
                stop=True,
            )
        atp3 = atp.rearrange("p (h e) -> p h e", h=H)
        rr = p_st.tile([128, H], F32, tag="rr", name="rr")
        nc.vector.reciprocal(out=rr, in_=atp3[:, :, E])
        sb = p_at.tile([128, H * E], BF16, tag=f"at{p}", name=f"at{p}")
        if STRIDE0:
            nc.vector.tensor_tensor(
                out=sb.rearrange("p (h e) -> p h e", h=H),
                in0=atp3[:, :, 0:E],
                in1=rr.broadcast_to([128, H, E]),
                op=OP.mult,
            )
        else:
            for h in range(H):
                nc.vector.tensor_scalar(
                    out=sb[:, h * E : (h + 1) * E],
                    in0=atp[:, h * EP : h * EP + E],
                    scalar1=rr[:, h : h + 1],
                    scalar2=None,
                    op0=OP.mult,
                )
        st.setdefault("at", []).append(sb)

    # ---------- attnT transposes (end of the tile's emission) ----------
    def stage_aT(st):
        st["aT"] = transpose_feat(st["at"], p_aT, "aT")

    # ---------- Wo + residual + LN2 (next emission; aT copies long done) ----
    def stage_wo_ln2(st):
        aT = st["aT"]
        hr_ts = []
        for c in range(TC):
            pm = ps.tile([128, D], F32, tag="ps", name="wo_ps")
            for k in range(KC):
                nc.tensor.matmul(
                    out=pm,
                    lhsT=aT[:, k * NT + c * 128 : k * NT + (c + 1) * 128],
                    rhs=wo[k],
                    start=(k == 0),
                    stop=(k == KC - 1),
                )
            hr = p_hr.tile([128, D], F32, tag="hr", name="hr")
            nc.vector.tensor_tensor(out=hr, in0=pm, in1=st["h"][c], op=OP.add)
            if flags["bo"]:
                nc.vector.tensor_tensor(out=hr, in0=hr, in1=bo_b, op=OP.add)
            hr_ts.append(hr)

        st["h2"] = [
            layernorm(hr_ts[c], g2_b, be2_b, flags["g2be2"], p_h2, "h2")
            for c in range(TC)
        ]

    # ---------- h2T transposes (next emission, LN2 long done) ----------
    def stage_h2T(st):
        st["h2T"] = transpose_feat(st["h2"], p_h2T, "h2T")

    # ---------- FFN1 (+relu) ----------
    def stage_ffn1(st):
        h2T = st["h2T"]
        rel = p_rel.tile([128, FC * NT], BF16, tag="rel", name="rel")
        nb1 = FC // 4 if BANK2 else FC // 2
        w1grp = 4 if BANK2 else 2
        for b4 in range(nb1):
            if BANK2:
                pm = ps2.tile([128, 4 * NT], F32, tag="ps2", name="f1_ps")
            else:
                pm = ps.tile([128, 2 * NT], F32, tag="ps", name="f1_ps")
            for fh in range(w1grp):
                f = w1grp * b4 + fh
                for k in range(KC):
                    nc.tensor.matmul(
                        out=pm[:, fh * NT : (fh + 1) * NT],
                        lhsT=w1[(k, f)],
                        rhs=h2T[:, k * NT : (k + 1) * NT],
                        start=(k == 0),
                        stop=(k == KC - 1),
                    )
            if flags["b1"]:
                for fh in range(w1grp):
                    f = w1grp * b4 + fh
                    nc.scalar.activation(
                        out=rel[:, f * NT : (f + 1) * NT],
                        in_=pm[:, fh * NT : (fh + 1) * NT],
                        func=AF.Relu,
                        bias=b1c[:, f : f + 1],
                        scale=1.0,
                    )
            elif b4 % 2 == 0:
                nc.scalar.activation(
                    out=rel[:, b4 * w1grp * NT : (b4 + 1) * w1grp * NT],
                    in_=pm,
                    func=AF.Relu,
                    bias=0.0,
                    scale=1.0,
                )
            else:
                nc.vector.tensor_relu(
                    out=rel[:, b4 * w1grp * NT : (b4 + 1) * w1grp * NT], in_=pm
                )
        st["rel"] = rel

    # ---------- FFN2 chunk + residual + store ----------
    def stage_ffn2_chunk(st, it, c):
        row0 = it * NT
        rel = st["rel"]
        pm = ps.tile([128, D], F32, tag="ps", name="f2_ps")
        for f in range(FC):
            nc.tensor.matmul(
                out=pm,
                lhsT=rel[:, f * NT + c * 128 : f * NT + (c + 1) * 128],
                rhs=w2[f],
                start=(f == 0),
                stop=(f == FC - 1),
            )
        o_t = p_out.tile([128, D], F32, tag="o", name="o")
        nc.vector.tensor_tensor(out=o_t, in0=pm, in1=st["h2"][c], op=OP.add)
        if flags["b2"]:
            nc.vector.tensor_tensor(out=o_t, in0=o_t, in1=b2_b, op=OP.add)
        nc.sync.dma_start(
            out=out_dr[row0 + c * 128 : row0 + (c + 1) * 128, :], in_=o_t
        )

    if PIPELINE:
        # 4-deep software pipeline; every PE stage placed so its cross-
        # engine dependencies resolved during earlier PE work:
        #   ln1(j+1) [no PE] | qkv(j) [deps one emission old] |
        #   wo+hr+LN2(j-1) [aT copies aged during qkv] | sc(j,p0) |
        #   h2T(j-2) [LN2(j-2) a full emission old] | sc(j,p1) |
        #   ffn1(j-2) [h2T copy covered by sc p1] | ffn2a(j-2) |
        #   atv(j,p0) atv(j,p1) [exp/mask aged during ffn] |
        #   ffn2b(j-2) [covers at-packs] | aT(j) transposes
        states = {}
        for j in range(n_tiles + 3):
            t_ln = j + 1       # LN1 (and, at emission end, hT+QKV)
            t_at = j           # scores + attnV + aT transposes
            t_wo = j - 1       # wo + LN2 + h2T + FFN1
            t_f2 = j - 2       # FFN2 + store
            if j == 0 and n_tiles > 0:
                states[0] = stage_ln1(0)
                stage_hT(states[0])
                stage_qkv(states[0])
            if t_ln < n_tiles:
                states[t_ln] = stage_ln1(t_ln)
            if 0 <= t_wo < n_tiles:
                stw = states[t_wo]
                stage_wo_ln2(stw)
            if t_at < n_tiles:
                sta = states[t_at]
                stage_scores_pair(sta, 0, "all")
                stage_scores_pair(sta, 1, "all")
            if 0 <= t_f2 < n_tiles:
                stf = states[t_f2]
                stage_ffn2_chunk(stf, t_f2, 0)
                stage_ffn2_chunk(stf, t_f2, 1)
                del states[t_f2]
            if 0 <= t_wo < n_tiles:
                stage_h2T(stw)
                stage_ffn1(stw)
            if t_at < n_tiles:
                stage_atv_pair(sta, 0)
                stage_atv_pair(sta, 1)
            if t_ln < n_tiles:
                stage_hT(states[t_ln])
            if t_at < n_tiles:
                stage_aT(sta)
            if t_ln < n_tiles:
                stage_qkv(states[t_ln])
    else:
        def dummy_out(it):
            row0 = it * NT
            for c in range(TC):
                o_t = p_out.tile([128, D], F32, tag="o", name="o")
                nc.vector.memset(o_t, 0.0)
                nc.sync.dma_start(
                    out=out_dr[row0 + c * 128 : row0 + (c + 1) * 128, :], in_=o_t
                )

        for it in range(n_tiles):
            st = stage_ln1(it)
            if CUT >= 2:
                stage_hT(st)
                stage_qkv(st)
            if CUT >= 3:
                stage_scores_pair(st, 0)
                stage_scores_pair(st, 1)
            if CUT >= 4:
                stage_atv_pair(st, 0)
                stage_atv_pair(st, 1)
            if CUT >= 5:
                stage_aT(st)
                stage_wo_ln2(st)
            if CUT >= 6:
                stage_h2T(st)
                stage_ffn1(st)
            if CUT >= 7:
                stage_ffn2_chunk(st, it, 0)
                stage_ffn2_chunk(st, it, 1)
            else:
                dummy_out(it)

    ctx.close()


def prep_inputs(inputs, b_core):
    import ml_dtypes

    f32 = np.float32
    bf16 = ml_dtypes.bfloat16
    wq, wk, wvv = (np.asarray(inputs[k], f32) for k in ("wq", "wk", "wv"))
    bq, bk, bv = (np.asarray(inputs[k], f32) for k in ("bq", "bk", "bv"))
    wo, bo = np.asarray(inputs["wo"], f32), np.asarray(inputs["bo"], f32)
    w1, b1 = np.asarray(inputs["w1"], f32), np.asarray(inputs["b1"], f32)
    w2, b2 = np.asarray(inputs["w2"], f32), np.asarray(inputs["b2"], f32)
    g1, be1 = np.asarray(inputs["g1"], f32), np.asarray(inputs["be1"], f32)
    g2, be2 = np.asarray(inputs["g2"], f32), np.asarray(inputs["be2"], f32)

    # wqk[qi, k, ch] = [128, 128]: cols 0:48 head 2ch, 64:112 head 2ch+1
    wqk = np.zeros((2, KC, 4, 128, 128), f32)
    for qi, w in enumerate((wq, wk)):
        for k in range(KC):
            for ch in range(4):
                wqk[qi, k, ch, :, 0:E] = w[2 * ch][k * 128 : (k + 1) * 128, :]
                wqk[qi, k, ch, :, EP : EP + E] = w[2 * ch + 1][k * 128 : (k + 1) * 128, :]
    wqz = np.zeros((H, KC, 128, 128), f32)
    for h in range(H):
        for k in range(KC):
            wqz[h, k, :, (h % 2) * EP : (h % 2) * EP + E] = wq[h][
                k * 128 : (k + 1) * 128, :
            ]
    # cols 0..7: per-head q bias; cols 8..11: per-chunk k bias
    bqk = np.zeros((128, 12), f32)
    for h in range(H):
        bqk[(h % 2) * EP : (h % 2) * EP + E, h] = bq[h]
    for ch in range(4):
        bqk[0:E, 8 + ch] = bk[2 * ch]
        bqk[EP : EP + E, 8 + ch] = bk[2 * ch + 1]

    # wv packed: [KC, 128, 392] cols h*49+e, ones slot at h*49+48 (zeros)
    wv_p = np.zeros((KC, 128, H * (E + 1)), f32)
    for k in range(KC):
        for h in range(H):
            wv_p[k, :, h * (E + 1) : h * (E + 1) + E] = wvv[h][
                k * 128 : (k + 1) * 128, :
            ]
    bv_b = np.zeros((H * (E + 1),), f32)
    for h in range(H):
        bv_b[h * (E + 1) : h * (E + 1) + E] = bv[h]

    # wo chunks: [KC, 128, D], rows PACKED (h*48) to match attnT layout
    wo_c = np.stack([wo[k * 128 : (k + 1) * 128, :] for k in range(KC)])

    w1_c = np.zeros((KC, FC, 128, 128), f32)
    for k in range(KC):
        for f in range(FC):
            w1_c[k, f] = w1[k * 128 : (k + 1) * 128, f * 128 : (f + 1) * 128]
    b1c = np.zeros((128, FC), f32)
    for f in range(FC):
        b1c[:, f] = b1[f * 128 : (f + 1) * 128]
    w2_c = np.stack([w2[f * 128 : (f + 1) * 128, :] for f in range(FC)])

    # maskT[s, h*128+q]: same batch (s//64==q//64) and causal (s%64 <= q%64)
    s_i = np.arange(128)[:, None]
    q_i = np.arange(128)[None, :]
    base = ((s_i // T) == (q_i // T)) & ((s_i % T) <= (q_i % T))
    maskT = np.tile(base.astype(f32), (1, H))

    bcast = lambda v, w: np.broadcast_to(v[None, :], (128, w)).copy()

    flags = {
        "g1be1": bool(np.any(g1 != 1) or np.any(be1 != 0)),
        "g2be2": bool(np.any(g2 != 1) or np.any(be2 != 0)),
        "bqk": bool(np.any(bq) or np.any(bk)),
        "bv": bool(np.any(bv)),
        "bo": bool(np.any(bo)),
        "b1": bool(np.any(b1)),
        "b2": bool(np.any(b2)),
    }
    common = dict(
        ident=np.eye(128, dtype=bf16),
        maskT=maskT.astype(bf16),
        wqk=wqk.astype(bf16),
        wqz=wqz.astype(bf16),
        wv=wv_p.astype(bf16),
        wo=wo_c.astype(bf16),
        w1=w1_c.astype(bf16),
        w2=w2_c.astype(bf16),
        bqk=bqk,
        bv_b=bcast(bv_b, DP),
        b1c=b1c,
        g1_b=bcast(g1, D),
        be1_b=bcast(be1, D),
        g2_b=bcast(g2, D),
        be2_b=bcast(be2, D),
        bo_b=bcast(bo, D),
        b2_b=bcast(b2, D),
    )
    return common, flags


CONST_SHAPES = dict(
    ident=(128, 128),
    maskT=(128, H * 128),
    wqk=(2, KC, 4, 128, 128),
    wqz=(H, KC, 128, 128),
    wv=(KC, 128, H * (E + 1)),
    wo=(KC, 128, D),
    w1=(KC, FC, 128, 128),
    w2=(FC, 128, D),
    bqk=(128, 12),
    bv_b=(128, H * (E + 1)),
    b1c=(128, FC),
    g1_b=(128, D),
    be1_b=(128, D),
    g2_b=(128, D),
    be2_b=(128, D),
    bo_b=(128, D),
    b2_b=(128, D),
)


WEIGHT_NAMES = {"ident", "maskT", "wqk", "wqz", "wv", "wo", "w1", "w2"}


def _make_bacc():
    """Bacc whose act-table placement is steered to the single table that
    serves every activation func this kernel uses (ln, exp, relu, copy,
    identity all live in natural_log_exp_and_others). The default greedy
    choice ping-pongs ln->natural_log / exp->exp_and_others, costing a
    1283ns ACT_TABLE_LOAD 6x per tile. Stripping our funcs from every
    OTHER table (list order, hence act_func_set_id, unchanged) forces the
    pass to pick the shared table once; the emitted id still names a real
    table containing all used funcs, so HW semantics are unchanged."""
    from concourse import bacc
    from concourse.hw_specs import get_activation_tables

    AF = mybir.ActivationFunctionType
    ours = {AF.Ln, AF.Exp, AF.Relu, AF.Copy, AF.Identity}
    target = "natural_log_exp_and_others"

    class _Bacc(bacc.Bacc):
        def insert_act_table_loads(self):
            import bass_rust as _bass_rust

            has_activation = any(
                isinstance(i, mybir.InstActivation)
                for b in self.main_func.blocks
                for i in b.instructions
            )
            if not has_activation:
                return
            tables = [
                (nm, (fs if nm == target else (set(fs) - ours)))
                for nm, fs in get_activation_tables(self.m.arch).items()
            ]
            _bass_rust.insert_act_table_loads(self, tables)

    if not ACT_TRICK:
        return bacc.Bacc("TRN2", target_bir_lowering=False, debug=False)
    return _Bacc("TRN2", target_bir_lowering=False, debug=False)


def build_program(b_core, flags):
    nc = _make_bacc()
    aps = {
        name: nc.dram_tensor(
            name, list(sh), BF16 if name in WEIGHT_NAMES else F32,
            kind="ExternalInput",
        ).ap()
        for name, sh in {**CONST_SHAPES, "x": (b_core, T, D)}.items()
    }
    aps["out"] = nc.dram_tensor("out", [b_core, T, D], F32, kind="ExternalOutput").ap()
    aps["flags"] = flags
    with tile.TileContext(nc) as tc:
        build_body(tc, aps, b_core)
    nc.compile()
    return nc


def kernel(**inputs):
    from concourse.bass_utils import run_bass_kernel_spmd

    x = np.ascontiguousarray(np.asarray(inputs["x"], np.float32))
    common, flags = prep_inputs(inputs, B_CORE)
    nc = build_program(B_CORE, flags)
    in_maps = []
    for c in range(N_CORES):
        m = dict(common)
        m["x"] = np.ascontiguousarray(x[c * B_CORE : (c + 1) * B_CORE])
        in_maps.append(m)
    res = run_bass_kernel_spmd(nc, in_maps, core_ids=list(range(N_CORES)))
    out = np.concatenate([r["out"] for r in res.results], axis=0)
    return out.astype(np.float32)
